# revision 1
# baseline (speedup 1.0000x reference)
"""Trainium2 Bass kernel for nn_Compressor (consecutive-run mean-pool compressor).

Semantics per batch element (T=4096, D=1024, blank_idx=0):
  - split preds[b] into consecutive runs
  - out[s] = mean(x[start_s : start_s+count_s]) for run s, zeroed when the
    run's label == 0; rows s >= n_runs stay zero (padding to T rows).

Strategy (pure data parallel, 2 examples per core on 8 cores):
  preds is tiny (256 KB) -> run-boundary metadata is computed on host with
  numpy and shipped to the device as small index/weight tensors. The heavy
  512 MB of x traffic is done on-device:
    - for each 128-row output tile: indirect-DMA gather of the 128 run-start
      rows of x, scaled per-partition by w_s = (label!=0)/count_s
    - runs with count>1 contribute their remaining rows via a second, small
      indirect gather (<= EMAX rows) folded in with a tiny [EMAX,128] matmul
      through PSUM
    - one 512 KB direct DMA store per output tile.
"""

import numpy as np

B, T, D = 16, 4096, 1024
N_CORES = 8
EX_PER_CORE = B // N_CORES  # 2
ROWS = EX_PER_CORE * T  # 8192 rows of x / out per core
NT = ROWS // 128  # 64 output tiles per core
BLANK_IDX = 0

_BUILD_CACHE: dict = {}


def _build(emax: int):
    """Build + compile the (input-independent) Bass kernel for a given EMAX."""
    import concourse.bass as bass
    import concourse.tile as tile
    from concourse import bacc, mybir

    nc = bacc.Bacc(
        "TRN2", target_bir_lowering=False, debug=False, enable_asserts=False
    )
    f32, i32 = mybir.dt.float32, mybir.dt.int32

    x_d = nc.dram_tensor("x", [ROWS, D], f32, kind="ExternalInput").ap()
    bidx_d = nc.dram_tensor("bidx", [128, NT], i32, kind="ExternalInput").ap()
    bw_d = nc.dram_tensor("bw", [128, NT], f32, kind="ExternalInput").ap()
    eidx_d = nc.dram_tensor("eidx", [emax, NT], i32, kind="ExternalInput").ap()
    esel_d = nc.dram_tensor("esel", [emax, NT * 128], f32, kind="ExternalInput").ap()
    out_d = nc.dram_tensor("out", [ROWS, D], f32, kind="ExternalOutput").ap()

    with tile.TileContext(nc) as tc:
        with (
            tc.tile_pool(name="const", bufs=1) as cpool,
            tc.tile_pool(name="gather", bufs=4) as gpool,
            tc.tile_pool(name="extras", bufs=4) as epool,
            tc.tile_pool(name="outp", bufs=4) as opool,
            tc.tile_pool(name="psum", bufs=3, space="PSUM") as ppool,
        ):
            idx_sb = cpool.tile([128, NT], i32)
            nc.sync.dma_start(idx_sb[:], bidx_d[:])
            w_sb = cpool.tile([128, NT], f32)
            nc.sync.dma_start(w_sb[:], bw_d[:])
            eidx_sb = cpool.tile([emax, NT], i32)
            nc.sync.dma_start(eidx_sb[:], eidx_d[:])
            esel_sb = cpool.tile([emax, NT * 128], f32)
            nc.sync.dma_start(esel_sb[:], esel_d[:])

            for j in range(NT):
                g = gpool.tile([128, D], f32)
                nc.gpsimd.indirect_dma_start(
                    out=g[:],
                    out_offset=None,
                    in_=x_d[:],
                    in_offset=bass.IndirectOffsetOnAxis(ap=idx_sb[:, j : j + 1], axis=0),
                )
                ge = epool.tile([emax, D], f32)
                nc.gpsimd.indirect_dma_start(
                    out=ge[:],
                    out_offset=None,
                    in_=x_d[:],
                    in_offset=bass.IndirectOffsetOnAxis(
                        ap=eidx_sb[:, j : j + 1], axis=0
                    ),
                )
                ps = ppool.tile([128, D], f32)
                nc.tensor.matmul(
                    out=ps[:, 0:512],
                    lhsT=esel_sb[:, j * 128 : (j + 1) * 128],
                    rhs=ge[:, 0:512],
                    start=True,
                    stop=True,
                )
                nc.tensor.matmul(
                    out=ps[:, 512:1024],
                    lhsT=esel_sb[:, j * 128 : (j + 1) * 128],
                    rhs=ge[:, 512:1024],
                    start=True,
                    stop=True,
                )
                o = opool.tile([128, D], f32)
                nc.vector.tensor_scalar_mul(
                    out=o[:], in0=g[:], scalar1=w_sb[:, j : j + 1]
                )
                nc.vector.tensor_tensor(
                    out=o[:], in0=o[:], in1=ps[:], op=mybir.AluOpType.add
                )
                nc.sync.dma_start(out_d[j * 128 : (j + 1) * 128, :], o[:])

    nc.compile()
    return nc


def _get_built(emax: int):
    if emax not in _BUILD_CACHE:
        _BUILD_CACHE[emax] = _build(emax)
    return _BUILD_CACHE[emax]


def _preprocess_example(p: np.ndarray):
    """p: [T] int32 -> (bidx [T] int32, bw [T] f32, extras list of
    (tile_local, src_row, dst_local, weight))."""
    change = np.empty(T, dtype=bool)
    change[0] = True
    change[1:] = p[1:] != p[:-1]
    starts = np.flatnonzero(change)  # [r]
    r = starts.size
    counts = np.diff(np.append(starts, T))  # [r]
    labels = p[starts]
    valid = labels != BLANK_IDX
    w = np.where(valid, 1.0 / counts, 0.0).astype(np.float32)

    bidx = np.zeros(T, dtype=np.int32)
    bw = np.zeros(T, dtype=np.float32)
    bidx[:r] = starts.astype(np.int32)
    bw[:r] = w

    extras = []
    for s in np.flatnonzero(valid & (counts > 1)):
        for k in range(1, int(counts[s])):
            extras.append((s // 128, int(starts[s]) + k, int(s % 128), float(w[s])))
    return bidx, bw, extras


def _make_inputs(x: np.ndarray, preds: np.ndarray):
    """Full inputs -> (in_maps per core, emax)."""
    per_ex = [_preprocess_example(np.asarray(preds[b])) for b in range(B)]

    # global EMAX (max extras per 128-row output tile across all cores)
    emax = 4
    tiles_per_ex = T // 128
    for b in range(B):
        cnt = np.zeros(tiles_per_ex, dtype=np.int64)
        for tl, _, _, _ in per_ex[b][2]:
            cnt[tl] += 1
        if cnt.size:
            emax = max(emax, int(cnt.max()))
    emax = int((emax + 3) // 4 * 4)  # round up, keep tiny

    in_maps = []
    for c in range(N_CORES):
        b0, b1 = EX_PER_CORE * c, EX_PER_CORE * c + 1
        bidx = np.concatenate(
            [per_ex[b0][0], per_ex[b1][0] + T]
        )  # [8192] row idx into core's x
        bw = np.concatenate([per_ex[b0][1], per_ex[b1][1]])
        bidx_t = np.ascontiguousarray(bidx.reshape(NT, 128).T).astype(np.int32)
        bw_t = np.ascontiguousarray(bw.reshape(NT, 128).T).astype(np.float32)

        eidx = np.zeros((emax, NT), dtype=np.int32)
        esel = np.zeros((emax, NT * 128), dtype=np.float32)
        fill = np.zeros(NT, dtype=np.int64)
        for e, b in enumerate((b0, b1)):
            for tl, src, dst, wt in per_ex[b][2]:
                jg = e * tiles_per_ex + tl
                i = fill[jg]
                fill[jg] += 1
                eidx[i, jg] = src + e * T
                esel[i, jg * 128 + dst] = wt

        xc = np.ascontiguousarray(
            np.asarray(x[b0 : b1 + 1], dtype=np.float32).reshape(ROWS, D)
        )
        in_maps.append(
            {"x": xc, "bidx": bidx_t, "bw": bw_t, "eidx": eidx, "esel": esel}
        )
    return in_maps, emax


def _run(in_maps, emax, trace=False):
    from concourse.bass_utils import run_bass_kernel_spmd

    nc = _get_built(emax)
    return run_bass_kernel_spmd(nc, in_maps, list(range(N_CORES)), trace=trace)


def kernel(x: np.ndarray, preds: np.ndarray) -> np.ndarray:
    x = np.asarray(x)
    preds = np.asarray(preds)
    in_maps, emax = _make_inputs(x, preds)
    res = _run(in_maps, emax)
    out = np.empty((B, T, D), dtype=np.float32)
    for c in range(N_CORES):
        oc = res.results[c]["out"].reshape(EX_PER_CORE, T, D)
        out[EX_PER_CORE * c : EX_PER_CORE * (c + 1)] = oc
    return out


# revision 2
# speedup vs baseline: 1.0597x; 1.0597x over previous
"""Trainium2 Bass kernel for nn_Compressor (consecutive-run mean-pool compressor).

Semantics per batch element (T=4096, D=1024, blank_idx=0):
  - split preds[b] into consecutive runs
  - out[s] = mean(x[start_s : start_s+count_s]) for run s, zeroed when the
    run's label == 0; rows s >= n_runs stay zero (padding to T rows).

Strategy (pure data parallel, 2 examples per core on 8 cores):
  preds is tiny (256 KB) -> run metadata is computed on host with numpy and
  shipped as small index/weight tensors; the 512 MB of x moves on-device.

  Since preds is uniform over 32 labels, ~97% of runs have length 1, so the
  output is nearly a row-gather of x. Per 512 output rows (one block-tile):
    - ONE indirect DMA gathers 128 blocks of L=4 consecutive x rows (16 KB
      per descriptor - keeps the SWDGE descriptor count low). Block anchors
      are majority-voted on host so ~97% of output rows are covered by the
      block slot at their position.
    - 4 per-slot DVE multiplies apply w_s = (label!=0)/count_s (0 for
      blank/padding/mismatched rows).
    - ONE blocked 2 MB store (16 KB per descriptor).
  Rows not covered by their slot (anchor mismatch) and rows of runs with
  count>1 get their remaining contributions via compacted single-row
  gathers + weight + indirect scatter-add (DMA CCE accumulate) into the
  output DRAM, after a barrier; entries hitting the same output row are
  split into dependency-chained phases so RMW never races.
"""

import numpy as np

B, T, D = 16, 4096, 1024
N_CORES = 8
EX_PER_CORE = B // N_CORES  # 2
ROWS = EX_PER_CORE * T  # 8192 rows of x / out per core
L = 4  # rows per gather/store block
NTB = ROWS // (128 * L)  # 16 block-tiles per core
BLANK_IDX = 0

_BUILD_CACHE: dict = {}


def _build(groups_per_phase: tuple):
    """Build + compile the Bass kernel. groups_per_phase[p] = number of
    128-entry scatter-add groups in phase p (phases are dep-chained)."""
    import concourse.bass as bass
    import concourse.tile as tile
    from concourse import bacc, mybir
    from concourse.tile import add_dep_helper

    n_groups = int(sum(groups_per_phase))

    nc = bacc.Bacc(
        "TRN2", target_bir_lowering=False, debug=False, enable_asserts=False
    )
    f32, i32 = mybir.dt.float32, mybir.dt.int32

    x_d = nc.dram_tensor("x", [ROWS, D], f32, kind="ExternalInput").ap()
    bidx_d = nc.dram_tensor("bidx", [128, NTB], i32, kind="ExternalInput").ap()
    bw_d = nc.dram_tensor("bw", [128, NTB * L], f32, kind="ExternalInput").ap()
    out_d = nc.dram_tensor("out", [ROWS, D], f32, kind="ExternalOutput").ap()
    if n_groups:
        eq_d = nc.dram_tensor("eq", [128, n_groups], i32, kind="ExternalInput").ap()
        ew_d = nc.dram_tensor("ew", [128, n_groups], f32, kind="ExternalInput").ap()
        ed_d = nc.dram_tensor("ed", [128, n_groups], i32, kind="ExternalInput").ap()

    with tile.TileContext(nc) as tc:
        with (
            tc.tile_pool(name="const", bufs=1) as cpool,
            tc.tile_pool(name="gather", bufs=4) as gpool,
            tc.tile_pool(name="extras", bufs=8) as epool,
        ):
            idx_sb = cpool.tile([128, NTB], i32)
            nc.sync.dma_start(idx_sb[:], bidx_d[:])
            w_sb = cpool.tile([128, NTB * L], f32)
            nc.sync.dma_start(w_sb[:], bw_d[:])
            if n_groups:
                eq_sb = cpool.tile([128, n_groups], i32)
                nc.sync.dma_start(eq_sb[:], eq_d[:])
                ew_sb = cpool.tile([128, n_groups], f32)
                nc.sync.dma_start(ew_sb[:], ew_d[:])
                ed_sb = cpool.tile([128, n_groups], i32)
                nc.sync.dma_start(ed_sb[:], ed_d[:])

            # main loop: blocked gather -> per-slot scale -> blocked store
            for j in range(NTB):
                g = gpool.tile([128, L * D], f32)
                nc.gpsimd.indirect_dma_start(
                    out=g[:],
                    out_offset=None,
                    in_=x_d[:],
                    in_offset=bass.IndirectOffsetOnAxis(
                        ap=idx_sb[:, j : j + 1], axis=0
                    ),
                )
                for l in range(L):
                    nc.vector.tensor_scalar_mul(
                        out=g[:, l * D : (l + 1) * D],
                        in0=g[:, l * D : (l + 1) * D],
                        scalar1=w_sb[:, j * L + l : j * L + l + 1],
                    )
                nc.sync.dma_start(
                    out_d[j * 128 * L : (j + 1) * 128 * L, :].rearrange(
                        "(p l) d -> p (l d)", l=L
                    ),
                    g[:],
                )

            # extras: gather single rows + scale (overlaps the main loop)
            ge_tiles = []
            for gi in range(n_groups):
                ge = epool.tile([128, D], f32)
                nc.gpsimd.indirect_dma_start(
                    out=ge[:],
                    out_offset=None,
                    in_=x_d[:],
                    in_offset=bass.IndirectOffsetOnAxis(
                        ap=eq_sb[:, gi : gi + 1], axis=0
                    ),
                )
                nc.vector.tensor_scalar_mul(
                    out=ge[:], in0=ge[:], scalar1=ew_sb[:, gi : gi + 1]
                )
                ge_tiles.append(ge)

            if n_groups:
                # all base stores must land before any scatter-add RMW
                tc.strict_bb_all_engine_barrier()
                gi = 0
                prev_phase_insts = []
                for ph, cnt in enumerate(groups_per_phase):
                    phase_insts = []
                    for _ in range(int(cnt)):
                        inst = nc.gpsimd.indirect_dma_start(
                            out=out_d[:],
                            out_offset=bass.IndirectOffsetOnAxis(
                                ap=ed_sb[:, gi : gi + 1], axis=0
                            ),
                            in_=ge_tiles[gi][:],
                            in_offset=None,
                            compute_op=mybir.AluOpType.add,
                        )
                        for prev in prev_phase_insts:
                            add_dep_helper(inst.ins, prev.ins)
                        phase_insts.append(inst)
                        gi += 1
                    prev_phase_insts = phase_insts

    nc.compile()
    return nc


def _get_built(groups_per_phase: tuple):
    key = tuple(groups_per_phase)
    if key not in _BUILD_CACHE:
        _BUILD_CACHE[key] = _build(key)
    return _BUILD_CACHE[key]


def _preprocess_example(p: np.ndarray):
    """p: [T] int32 -> per-output-row run data (g, c, w) and per-block
    majority anchors + slot weights; extras as (dst_row, src_row, w) lists
    indexed by phase."""
    change = np.empty(T, dtype=bool)
    change[0] = True
    change[1:] = p[1:] != p[:-1]
    starts = np.flatnonzero(change)
    r = starts.size
    counts = np.diff(np.append(starts, T))
    labels = p[starts]
    w = np.where(labels != BLANK_IDX, 1.0 / counts, 0.0).astype(np.float32)

    g = np.zeros(T, dtype=np.int64)
    c = np.ones(T, dtype=np.int64)
    wv = np.zeros(T, dtype=np.float32)
    g[:r] = starts
    c[:r] = counts
    wv[:r] = w

    n_blocks = T // L
    anchors = np.zeros(n_blocks, dtype=np.int64)
    wslot = np.zeros(T, dtype=np.float32)
    extras = []  # (o, src_local, weight, k_order)

    lvec = np.arange(L)
    votes_all = g.reshape(n_blocks, L) - lvec  # [n_blocks, L]
    valid_all = wv.reshape(n_blocks, L) > 0

    for bk in range(n_blocks):
        valid = valid_all[bk]
        if not valid.any():
            anchors[bk] = 0
            continue
        votes = votes_all[bk][valid]
        vals, cnts = np.unique(votes, return_counts=True)
        q = int(vals[np.argmax(cnts)])
        q = min(q, T - L)
        anchors[bk] = q
        o0 = bk * L
        for l in range(L):
            o = o0 + l
            if wv[o] <= 0:
                continue
            matched = g[o] == q + l
            if matched:
                wslot[o] = wv[o]
                ks = range(1, int(c[o]))
            else:
                ks = range(0, int(c[o]))
            for ko, k in enumerate(ks):
                extras.append((o, int(g[o]) + k, float(wv[o]), ko))
    return anchors, wslot, extras


def _make_inputs(x: np.ndarray, preds: np.ndarray):
    """Full inputs -> (in_maps per core, groups_per_phase)."""
    per_ex = [_preprocess_example(np.asarray(preds[b])) for b in range(B)]

    # phase structure must be uniform across cores (single NEFF):
    # phase p gets the p-th extra of each output row; group counts are the
    # max over cores, padded with null entries (w=0, dst=ROWS-1).
    core_phase_entries = []  # [core][phase] -> list of (src, dst, w)
    max_phases = 0
    for cidx in range(N_CORES):
        phases: list = []
        for e in range(EX_PER_CORE):
            b = EX_PER_CORE * cidx + e
            off = e * T
            for o, src, wt, ko in per_ex[b][2]:
                while len(phases) <= ko:
                    phases.append([])
                phases[ko].append((src + off, o + off, wt))
        core_phase_entries.append(phases)
        max_phases = max(max_phases, len(phases))

    groups_per_phase = []
    for ph in range(max_phases):
        most = max(
            len(phs[ph]) if ph < len(phs) else 0 for phs in core_phase_entries
        )
        groups_per_phase.append((most + 127) // 128)
    groups_per_phase = tuple(int(gp) for gp in groups_per_phase if gp > 0)
    n_groups = int(sum(groups_per_phase))

    in_maps = []
    for cidx in range(N_CORES):
        b0 = EX_PER_CORE * cidx
        bidx = np.zeros((128, NTB), dtype=np.int32)
        bw = np.zeros((128, NTB * L), dtype=np.float32)
        for e in range(EX_PER_CORE):
            anchors, wslot, _ = per_ex[b0 + e]
            # block bk of example e -> tile j = (e*T + bk*L) // (128*L),
            # partition p = (bk*L % (128*L)) // L
            for bk in range(T // L):
                orow = e * T + bk * L
                j = orow // (128 * L)
                prt = (orow % (128 * L)) // L
                bidx[prt, j] = anchors[bk] + e * T
                bw[prt, j * L : (j + 1) * L] = wslot[bk * L : (bk + 1) * L]

        eq = np.zeros((128, n_groups), dtype=np.int32)
        ew = np.zeros((128, n_groups), dtype=np.float32)
        ed = np.full((128, n_groups), ROWS - 1, dtype=np.int32)
        gbase = 0
        phases = core_phase_entries[cidx]
        for ph, gp in enumerate(groups_per_phase):
            ents = phases[ph] if ph < len(phases) else []
            for i, (src, dst, wt) in enumerate(ents):
                grp = gbase + i // 128
                prt = i % 128
                eq[prt, grp] = src
                ew[prt, grp] = wt
                ed[prt, grp] = dst
            gbase += gp

        xc = np.ascontiguousarray(
            np.asarray(x[b0 : b0 + EX_PER_CORE], dtype=np.float32).reshape(ROWS, D)
        )
        im = {"x": xc, "bidx": bidx, "bw": bw}
        if n_groups:
            im.update({"eq": eq, "ew": ew, "ed": ed})
        in_maps.append(im)
    return in_maps, groups_per_phase


def _run(in_maps, groups_per_phase, trace=False):
    from concourse.bass_utils import run_bass_kernel_spmd

    nc = _get_built(groups_per_phase)
    return run_bass_kernel_spmd(nc, in_maps, list(range(N_CORES)), trace=trace)


def kernel(x: np.ndarray, preds: np.ndarray) -> np.ndarray:
    x = np.asarray(x)
    preds = np.asarray(preds)
    in_maps, gpp = _make_inputs(x, preds)
    res = _run(in_maps, gpp)
    out = np.empty((B, T, D), dtype=np.float32)
    for c in range(N_CORES):
        oc = res.results[c]["out"].reshape(EX_PER_CORE, T, D)
        out[EX_PER_CORE * c : EX_PER_CORE * (c + 1)] = oc
    return out


# revision 4
# speedup vs baseline: 1.3327x; 1.2576x over previous
"""Trainium2 Bass kernel for nn_Compressor (consecutive-run mean-pool compressor).

Semantics per batch element (T=4096, D=1024, blank_idx=0):
  - split preds[b] into consecutive runs
  - out[s] = mean(x[start_s : start_s+count_s]) for run s, zeroed when the
    run's label == 0; rows s >= n_runs stay zero (padding to T rows).

Strategy (pure data parallel, 2 examples per core on 8 cores):
  preds is tiny (256 KB) -> run metadata is computed on host with numpy and
  shipped as small index/weight tensors; the 512 MB of x moves on-device.
  Each core's x shard gets one extra all-zero row appended (index ROWS)
  used as a null target for padding gathers.

  Since preds is uniform over 32 labels, ~97% of runs have length 1, so the
  output is nearly a row-gather of x. Per 512 output rows (one block-tile):
    - ONE indirect DMA gathers 128 blocks of L=4 consecutive x rows (16 KB
      per descriptor, keeping SWDGE descriptor generation cheap). Block
      anchors are majority-voted on host so ~97% of output rows are covered
      by the block slot at their position.
    - 4 per-slot DVE multiplies apply w_s = (label!=0)/count_s (0 for
      blank/padding/mismatched rows).
    - ONE blocked 2 MB store (16 KB per descriptor).
  Output rows not fully covered by their slot (anchor mismatch, or runs
  with count>1) get the missing contribution via a compact path: entries
  are grouped by destination row (<=128 unique dsts per group, sorted by
  descending row count); each group's row-sums are built with a plain
  gather plus accumulate-gathers (DMA CCE add into SBUF; exhausted rows
  point at the zero row), scaled once by w, then indirect scatter-added
  (CCE add into DRAM) after the base stores.
"""

import numpy as np

B, T, D = 16, 4096, 1024
N_CORES = 8
EX_PER_CORE = B // N_CORES  # 2
ROWS = EX_PER_CORE * T  # 8192 rows of out per core
XR = ROWS + 1  # x shard rows incl. trailing zero row
ZROW = ROWS  # index of the zero row
L = 4  # rows per gather/store block
NTB = ROWS // (128 * L)  # 16 block-tiles per core
BLANK_IDX = 0

_BUILD_CACHE: dict = {}


def _build(rounds_per_group: tuple):
    """Build + compile the Bass kernel. rounds_per_group[g] = number of
    gather rounds (1 plain + R-1 accumulate) for extras group g."""
    import concourse.bass as bass
    import concourse.tile as tile
    from concourse import bacc, mybir
    from concourse.tile import add_dep_helper

    n_groups = len(rounds_per_group)
    n_round_cols = int(sum(rounds_per_group))

    nc = bacc.Bacc(
        "TRN2", target_bir_lowering=False, debug=False, enable_asserts=False
    )
    f32, i32 = mybir.dt.float32, mybir.dt.int32

    x_d = nc.dram_tensor("x", [XR, D], f32, kind="ExternalInput").ap()
    bidx_d = nc.dram_tensor("bidx", [128, NTB], i32, kind="ExternalInput").ap()
    bw_d = nc.dram_tensor("bw", [128, NTB * L], f32, kind="ExternalInput").ap()
    out_d = nc.dram_tensor("out", [ROWS, D], f32, kind="ExternalOutput").ap()
    if n_groups:
        eq_d = nc.dram_tensor(
            "eq", [128, n_round_cols], i32, kind="ExternalInput"
        ).ap()
        ew_d = nc.dram_tensor("ew", [128, n_groups], f32, kind="ExternalInput").ap()
        ed_d = nc.dram_tensor("ed", [128, n_groups], i32, kind="ExternalInput").ap()

    with tile.TileContext(nc) as tc:
        with (
            tc.tile_pool(name="const", bufs=1) as cpool,
            tc.tile_pool(name="gather", bufs=6) as gpool,
            tc.tile_pool(name="extras", bufs=max(1, n_groups)) as epool,
        ):
            idx_sb = cpool.tile([128, NTB], i32)
            nc.sync.dma_start(idx_sb[:], bidx_d[:])
            w_sb = cpool.tile([128, NTB * L], f32)
            nc.sync.dma_start(w_sb[:], bw_d[:])
            if n_groups:
                eq_sb = cpool.tile([128, n_round_cols], i32)
                nc.sync.dma_start(eq_sb[:], eq_d[:])
                ew_sb = cpool.tile([128, n_groups], f32)
                nc.sync.dma_start(ew_sb[:], ew_d[:])
                ed_sb = cpool.tile([128, n_groups], i32)
                nc.sync.dma_start(ed_sb[:], ed_d[:])

            # main loop: blocked gather -> per-slot scale -> blocked store
            for j in range(NTB):
                g = gpool.tile([128, L * D], f32)
                nc.gpsimd.indirect_dma_start(
                    out=g[:],
                    out_offset=None,
                    in_=x_d[:],
                    in_offset=bass.IndirectOffsetOnAxis(
                        ap=idx_sb[:, j : j + 1], axis=0
                    ),
                )
                for l in range(L):
                    nc.vector.tensor_scalar_mul(
                        out=g[:, l * D : (l + 1) * D],
                        in0=g[:, l * D : (l + 1) * D],
                        scalar1=w_sb[:, j * L + l : j * L + l + 1],
                    )
                nc.sync.dma_start(
                    out_d[j * 128 * L : (j + 1) * 128 * L, :].rearrange(
                        "(p l) d -> p (l d)", l=L
                    ),
                    g[:],
                )

            # extras: per-group row sums (overlap the main loop)
            ge_tiles = []
            col = 0
            for gi in range(n_groups):
                ge = epool.tile([128, D], f32)
                for t in range(rounds_per_group[gi]):
                    nc.gpsimd.indirect_dma_start(
                        out=ge[:],
                        out_offset=None,
                        in_=x_d[:],
                        in_offset=bass.IndirectOffsetOnAxis(
                            ap=eq_sb[:, col : col + 1], axis=0
                        ),
                        compute_op=(
                            mybir.AluOpType.bypass if t == 0 else mybir.AluOpType.add
                        ),
                    )
                    col += 1
                nc.vector.tensor_scalar_mul(
                    out=ge[:], in0=ge[:], scalar1=ew_sb[:, gi : gi + 1]
                )
                ge_tiles.append(ge)

            if n_groups:
                # all base stores must land before any scatter-add RMW;
                # Tile also serializes the scatters among themselves (WAW
                # on out), the explicit deps are belt-and-suspenders.
                tc.strict_bb_all_engine_barrier()
                prev = None
                for gi in range(n_groups):
                    inst = nc.gpsimd.indirect_dma_start(
                        out=out_d[:],
                        out_offset=bass.IndirectOffsetOnAxis(
                            ap=ed_sb[:, gi : gi + 1], axis=0
                        ),
                        in_=ge_tiles[gi][:],
                        in_offset=None,
                        compute_op=mybir.AluOpType.add,
                    )
                    if prev is not None:
                        add_dep_helper(inst.ins, prev.ins)
                    prev = inst

    nc.compile()
    return nc


def _get_built(rounds_per_group: tuple):
    if rounds_per_group not in _BUILD_CACHE:
        _BUILD_CACHE[rounds_per_group] = _build(rounds_per_group)
    return _BUILD_CACHE[rounds_per_group]


def _preprocess_example(p: np.ndarray):
    """p: [T] int32 -> (anchors [T/L], wslot [T], extras dict
    dst_row -> (weight, [src rows]))."""
    change = np.empty(T, dtype=bool)
    change[0] = True
    change[1:] = p[1:] != p[:-1]
    starts = np.flatnonzero(change)
    r = starts.size
    counts = np.diff(np.append(starts, T))
    labels = p[starts]
    w = np.where(labels != BLANK_IDX, 1.0 / counts, 0.0).astype(np.float32)

    g = np.zeros(T, dtype=np.int64)
    c = np.ones(T, dtype=np.int64)
    wv = np.zeros(T, dtype=np.float32)
    g[:r] = starts
    c[:r] = counts
    wv[:r] = w

    n_blocks = T // L
    anchors = np.zeros(n_blocks, dtype=np.int64)
    wslot = np.zeros(T, dtype=np.float32)
    extras = {}

    lvec = np.arange(L)
    votes_all = g.reshape(n_blocks, L) - lvec
    valid_all = wv.reshape(n_blocks, L) > 0

    for bk in range(n_blocks):
        valid = valid_all[bk]
        if not valid.any():
            continue
        votes = votes_all[bk][valid]
        vals, cnts = np.unique(votes, return_counts=True)
        q = int(vals[np.argmax(cnts)])
        q = min(q, T - L)
        anchors[bk] = q
        o0 = bk * L
        for l in range(L):
            o = o0 + l
            if wv[o] <= 0:
                continue
            matched = g[o] == q + l
            if matched:
                wslot[o] = wv[o]
                ks = range(1, int(c[o]))
            else:
                ks = range(0, int(c[o]))
            rows = [int(g[o]) + k for k in ks]
            if rows:
                extras[o] = (float(wv[o]), rows)
    return anchors, wslot, extras


def _make_inputs(x: np.ndarray, preds: np.ndarray):
    """Full inputs -> (in_maps per core, rounds_per_group)."""
    per_ex = [_preprocess_example(np.asarray(preds[b])) for b in range(B)]

    # per core: entries (dst, w, [srcs]) sorted by descending len(srcs)
    core_entries = []
    for cidx in range(N_CORES):
        ents = []
        for e in range(EX_PER_CORE):
            b = EX_PER_CORE * cidx + e
            off = e * T
            for o, (wt, rows) in per_ex[b][2].items():
                ents.append((off + o, wt, [off + s for s in rows]))
        ents.sort(key=lambda t: -len(t[2]))
        core_entries.append(ents)

    # uniform group/round structure across cores (single NEFF)
    max_ents = max((len(e) for e in core_entries), default=0)
    n_groups = (max_ents + 127) // 128
    rounds_per_group = []
    for gi in range(n_groups):
        r = 1
        for ents in core_entries:
            grp = ents[gi * 128 : (gi + 1) * 128]
            if grp:
                r = max(r, max(len(t[2]) for t in grp))
        rounds_per_group.append(int(r))
    rounds_per_group = tuple(rounds_per_group)
    n_round_cols = int(sum(rounds_per_group))

    in_maps = []
    for cidx in range(N_CORES):
        b0 = EX_PER_CORE * cidx
        bidx = np.zeros((128, NTB), dtype=np.int32)
        bw = np.zeros((128, NTB * L), dtype=np.float32)
        for e in range(EX_PER_CORE):
            anchors, wslot, _ = per_ex[b0 + e]
            for bk in range(T // L):
                orow = e * T + bk * L
                j = orow // (128 * L)
                prt = (orow % (128 * L)) // L
                bidx[prt, j] = anchors[bk] + e * T
                bw[prt, j * L : (j + 1) * L] = wslot[bk * L : (bk + 1) * L]

        xc = np.empty((XR, D), dtype=np.float32)
        xc[:ROWS] = np.asarray(x[b0 : b0 + EX_PER_CORE], dtype=np.float32).reshape(
            ROWS, D
        )
        xc[ROWS] = 0.0
        im = {"x": xc, "bidx": bidx, "bw": bw}

        if n_groups:
            eq = np.full((128, n_round_cols), ZROW, dtype=np.int32)
            ew = np.zeros((128, n_groups), dtype=np.float32)
            ed = np.full((128, n_groups), ROWS - 1, dtype=np.int32)
            ents = core_entries[cidx]
            col = 0
            for gi, rounds in enumerate(rounds_per_group):
                grp = ents[gi * 128 : (gi + 1) * 128]
                for i, (dst, wt, srcs) in enumerate(grp):
                    ew[i, gi] = wt
                    ed[i, gi] = dst
                    for t, s in enumerate(srcs):
                        eq[i, col + t] = s
                col += rounds
            im.update({"eq": eq, "ew": ew, "ed": ed})
        in_maps.append(im)
    return in_maps, rounds_per_group


def _run(in_maps, rounds_per_group, trace=False):
    from concourse.bass_utils import run_bass_kernel_spmd

    nc = _get_built(rounds_per_group)
    return run_bass_kernel_spmd(nc, in_maps, list(range(N_CORES)), trace=trace)


def kernel(x: np.ndarray, preds: np.ndarray) -> np.ndarray:
    x = np.asarray(x)
    preds = np.asarray(preds)
    in_maps, rounds_per_group = _make_inputs(x, preds)
    res = _run(in_maps, rounds_per_group)
    out = np.empty((B, T, D), dtype=np.float32)
    for c in range(N_CORES):
        oc = res.results[c]["out"].reshape(EX_PER_CORE, T, D)
        out[EX_PER_CORE * c : EX_PER_CORE * (c + 1)] = oc
    return out
